# revision 43
# baseline (speedup 1.0000x reference)
"""Trainium2 Bass kernel for 3D conv: x[4,16,64,64,64] * w[16,16,3,3,3] + bias, pad=1.

Strategy (8 cores): shard over (batch, d-half) -> 8 shards of [16, 32+halo, 66, 66].
Per core, a "banded weight" matmul decomposition:
  - contraction K = (cin=16) x (d-window=8) = 128 partitions
  - output    M = (d_sub<=6 outputs) x (cout=16) <= 96 partitions
  - the 9 (kh,kw) taps are free-dim shifts over a zero-padded 66x66 (h,w) plane
  - lhsT[(cin,dw), (ds,co)] = W[co,cin,dw-ds,kh,kw] for 0<=dw-ds<=2 (banded, built on host)
Matmuls run in float32r (full-rate fp32 mode for free dim >= 256).
Bias is fused into the PSUM->SBUF extraction copy on the scalar engine.
"""

import os
from contextlib import ExitStack

import ml_dtypes
import numpy as np

import concourse.bass as bass
import concourse.mybir as mybir
import concourse.tile as tile
from concourse.bass_utils import run_bass_kernel_spmd

USE_BF16 = True            # bf16 x/weights/output (fp32 psum accumulation)
_MMDT = mybir.dt.bfloat16 if USE_BF16 else mybir.dt.float32r
_IODT = mybir.dt.bfloat16 if USE_BF16 else mybir.dt.float32
_NPDT = ml_dtypes.bfloat16 if USE_BF16 else np.float32

B, CIN, COUT, S = 4, 16, 16, 64
HP = WP = S + 2            # padded plane edge: 66
PLANE = HP * WP            # 4356
MARGIN = 68                # free-dim margin so shifted reads stay in-bounds
DSUB = 6                   # d outputs per full block
NDW = 8                    # d-window planes (DSUB + 2 halo)
DHALF = 32                 # output d planes per core
SHARD_D = DHALF + 6        # padded d planes per shard (windows span [-1, 37))
BLOCKS = [(0, 6), (6, 6), (12, 6), (18, 6), (24, 6), (30, 2)]
CROWS = 7                  # padded h-rows per psum chunk (7*66=462 <= 512 bank limit)
OBW = CROWS * S            # 448 output cols per full chunk

_nc_cache = None
LAST_RESULT = None         # BassKernelResults of the most recent run (for test.py)


def _strip_implied_waits(nc):
    """Remove semaphore waits that are transitively implied by another wait on
    the same instruction.

    Tile's add_semaphores emits the full non-transitive closure, so a matmul
    whose psum slot was last touched by (PE writes -> ACT read -> DVE memset)
    carries three waits — but walrus only supports a single sync-wait on a
    Matmult (fp32r matmuls are self-loading, and the wait rides the LDW
    struct). A wait (s >= v) is provably redundant if another wait on the
    same instruction targets a producer whose completion already implies
    (s >= v). We replay the scheduled instruction stream with vector clocks
    to compute each semaphore event's implied clock, then drop implied waits.

    In-order completion is assumed per compute-engine queue but NOT for DMA
    instructions (SDMA engines complete packets out of order), so DMA clocks
    only carry their own waits + update.
    """
    sem_count = {}
    sem_events = {}        # sem id -> list of (value_after, clock dict)
    engine_clock = {}
    engine_self = {}       # engine -> {sem id -> updates issued by that engine}

    def join(a, b):
        for k, v in b.items():
            if a.get(k, -1) < v:
                a[k] = v

    def snapshot(sid, val):
        for value_after, clk in sem_events.get(sid, ()):
            if value_after >= val:
                return clk
        return None

    for block in nc.m.functions[0].blocks:
        for inst in block.instructions:
            si = inst.sync_info
            if si is None:
                continue
            eng0 = str(inst.engine)
            is_dma0 = type(inst).__name__ in ("InstDMACopy", "InstDMATranspose")
            is_serial = (not is_dma0 and type(inst).__name__ not in
                         ("InstMatmult", "InstDrain", "InstEventSemaphore"))
            waits = list(si.on_wait)
            if is_serial and len(waits) > 1:
                # serial engines execute in order: a wait on the engine's own
                # completion semaphore for a value its predecessors already
                # produce is a no-op.
                own = engine_self.get(eng0, {})
                kept = [w for w in waits
                        if not (w.wait_mode == "sem-ge-imm"
                                and own.get(w.id, 0) >= w.wait_value)]
                if len(kept) < len(waits):
                    si.on_wait = kept
                    waits = kept
            snaps = []
            for w in waits:
                snaps.append(snapshot(w.id, w.wait_value)
                             if w.wait_mode == "sem-ge-imm" else None)
            if len(waits) > 1:
                keep = []
                for i, w in enumerate(waits):
                    if w.wait_mode != "sem-ge-imm":
                        keep.append(w)
                        continue
                    implied = False
                    for j, other in enumerate(waits):
                        if i == j or snaps[j] is None:
                            continue
                        if snaps[j].get(w.id, -1) >= w.wait_value:
                            implied = True
                            break
                    if not implied:
                        keep.append(w)
                if len(keep) < len(waits):
                    si.on_wait = keep
                    waits = keep
                    snaps = [snapshot(w.id, w.wait_value)
                             if w.wait_mode == "sem-ge-imm" else None
                             for w in waits]

            clk = {}
            for s in snaps:
                if s is not None:
                    join(clk, s)
            eng = str(inst.engine)
            is_dma = type(inst).__name__ in ("InstDMACopy", "InstDMATranspose")
            if not is_dma and eng in engine_clock:
                join(clk, engine_clock[eng])
            for u in si.on_update:
                if u.update_mode == "sem-add-imm":
                    sem_count[u.id] = sem_count.get(u.id, 0) + u.update_value
                elif u.update_mode == "sem-inc":
                    sem_count[u.id] = sem_count.get(u.id, 0) + 1
                else:
                    continue
                clk[u.id] = max(clk.get(u.id, 0), sem_count[u.id])
                sem_events.setdefault(u.id, []).append((sem_count[u.id], clk))
                if not is_dma:
                    es = engine_self.setdefault(eng, {})
                    es[u.id] = sem_count[u.id]
            if not is_dma:
                engine_clock[eng] = clk


def _build_nc():
    nc = bass.Bass()
    xs = nc.dram_tensor("xs", [CIN, SHARD_D, PLANE], _MMDT,
                        kind="ExternalInput")
    wb = nc.dram_tensor("wb", [128, 9 * 96], _MMDT,
                        kind="ExternalInput")
    bs = nc.dram_tensor("bs", [96, 1], mybir.dt.float32, kind="ExternalInput")
    out = nc.dram_tensor("out", [COUT, DHALF, S * S], _IODT,
                         kind="ExternalOutput")

    with ExitStack() as ctx:
        tc = ctx.enter_context(tile.TileContext(nc))
        consts = ctx.enter_context(tc.tile_pool(name="consts", bufs=1))
        xpool = ctx.enter_context(tc.tile_pool(name="xpool", bufs=6))
        opool = ctx.enter_context(tc.tile_pool(name="opool", bufs=2))
        pspool = ctx.enter_context(tc.tile_pool(name="pspool", bufs=7, space="PSUM"))

        shield = ctx.enter_context(tc.tile_pool(name="shield", bufs=1, space="PSUM"))
        sps = shield.tile([2, 8], mybir.dt.float32)
        ssb = consts.tile([1, 8], mybir.dt.float32)

        wtile = consts.tile([128, 9 * 96], _MMDT)
        nc.sync.dma_start(out=wtile, in_=wb[:, :])
        # prefetch all six block windows up front (SP ring, back to back)
        # split point between the two load halves of each block, in plane cols
        XSPLIT = 24 * WP  # rows [0,24) / [24,66): chunks 0-2 need only half A
        xts = []
        for blk, (dr0, dsc) in enumerate(BLOCKS):
            xt = xpool.tile([128, PLANE + 2 * MARGIN], _MMDT, tag="xt")
            # src iterates (cin, dw, plane) -> partition p = cin*8+dw.
            # All loads on the SP ring: FIFO order matches consumption order.
            # Each block in two halves so early chunks can start sooner.
            nc.sync.dma_start(
                out=xt[:, MARGIN:MARGIN + XSPLIT],
                in_=xs[:, dr0:dr0 + NDW, 0:XSPLIT],
            )
            nc.sync.dma_start(
                out=xt[:, MARGIN + XSPLIT:MARGIN + PLANE],
                in_=xs[:, dr0:dr0 + NDW, XSPLIT:PLANE],
            )
            xts.append(xt)
        btile = consts.tile([96, 1], mybir.dt.float32)
        nc.sync.dma_start(out=btile, in_=bs[:, :])
        ssv = consts.tile([1, 8], mybir.dt.float32)
        # walrus allows only one sync-wait on a Matmult; absorb each DMA's
        # completion wait with a dummy 2x2 PE / 1-elem DVE op reading the tile.
        nc.tensor.matmul(sps[0:2, 0:2], wtile[0:2, 0:2], wtile[0:2, 0:2],
                         start=True, stop=True)
        nc.vector.tensor_scalar_add(ssv[0:1, 0:1], btile[0:1, 0:1], 0.0)

        for blk, (dr0, dsc) in enumerate(BLOCKS):
            mv = 16 * dsc
            xt = xts[blk]
            # absorb the xt DMA waits (one per load half) on the PE engine
            nc.tensor.matmul(sps[0:2, 2:4], xt[0:2, MARGIN:MARGIN + 2],
                             xt[0:2, MARGIN:MARGIN + 2], start=True, stop=True)
            nc.tensor.matmul(
                sps[0:2, 4:6], xt[0:2, MARGIN + XSPLIT:MARGIN + XSPLIT + 2],
                xt[0:2, MARGIN + XSPLIT:MARGIN + XSPLIT + 2],
                start=True, stop=True)
            ob = opool.tile([96, S * S], _IODT, tag="ob")
            # absorb the ob-slot-release (out DMA) waits on the DVE engine
            # (extractions run on DVE); values are overwritten by extraction.
            nc.vector.memset(ob[0:1, 0:1], 0.0)
            nc.vector.memset(ob[0:1, 2048:2049], 0.0)
            nc.vector.memset(ob[0:1, 4032:4033], 0.0)
            # zero-wait sponges on the ACT queue for store-DMA lane waits
            nc.scalar.activation(ssb[0:1, 3:4], ssb[0:1, 0:1],
                                 mybir.ActivationFunctionType.Copy)
            nc.scalar.activation(ssb[0:1, 4:5], ssb[0:1, 1:2],
                                 mybir.ActivationFunctionType.Copy)
            for c in range(10):
                rc = CROWS if c < 9 else 1
                ncols = WP * rc
                ps = pspool.tile([96, 512], mybir.dt.float32, tag="ps")
                # absorb the psum-slot-release (ACT + transitive PE) waits on
                # DVE, so the chunk's first matmul carries at most one wait
                # (walrus allows a single sync-wait on Matmult).
                nc.vector.memset(ps[0:1, 0:1], 0.0)
                base = MARGIN + WP * (1 + CROWS * c)
                for t in range(9):
                    kh, kw = divmod(t, 3)
                    off = base + (kh - 1) * WP + (kw - 1)
                    nc.tensor.matmul(
                        ps[:mv, :ncols],
                        wtile[:, t * 96:t * 96 + mv],
                        xt[:, off:off + ncols],
                        start=(t == 0),
                        stop=(t == 8),
                    )
                if rc > 1:
                    src = ps[:mv, 1:1 + rc * WP].rearrange(
                        "p (r s) -> p r s", r=rc)[:, :, 0:S]
                    dst = ob[:mv, OBW * c:OBW * c + rc * S].rearrange(
                        "p (r s) -> p r s", r=rc)
                else:
                    src = ps[:mv, 1:1 + S]
                    dst = ob[:mv, OBW * c:OBW * c + S]
                # extraction on the (otherwise idle) vector engine: ACT copies
                # cost ~4x more per op and would serialize the psum recycle.
                nc.vector.tensor_scalar_add(dst, src, btile[:mv, :])
                if c in (4, 8):
                    # store finished columns as soon as their chunks extract;
                    # dest iterates (ds, co, cols) = partition ds*16+co.
                    # ACT-ring issue: stores never block SP-ring loads.
                    lo, hi = (0, 2048) if c == 4 else (2048, 4032)
                    osl = out[:, dr0:dr0 + dsc, lo:hi]
                    oap = bass.AP(tensor=osl.tensor, offset=osl.offset,
                                  ap=[osl.ap[1], osl.ap[0], osl.ap[2]])
                    nc.scalar.dma_start(out=oap, in_=ob[:mv, lo:hi])
            osl = out[:, dr0:dr0 + dsc, 4032:4096]
            oap = bass.AP(tensor=osl.tensor, offset=osl.offset,
                          ap=[osl.ap[1], osl.ap[0], osl.ap[2]])
            nc.scalar.dma_start(out=oap, in_=ob[:mv, 4032:4096])
    _strip_implied_waits(nc)
    _spread_adjacent_waits(nc)
    _push_waits_earlier(nc)
    _split_tail_drain_waits(nc)
    return nc


def _push_waits_earlier(nc):
    """For a DMACopy still carrying >1 waits (e.g. a DMAHW lane-ordering wait
    plus a data wait), move the extras onto an earlier zero-wait instruction
    of the same engine queue. Satisfying a wait earlier in the queue is
    strictly more conservative, hence always safe."""
    for block in nc.m.functions[0].blocks:
        insts = list(block.instructions)
        for idx, inst in enumerate(insts):
            si = inst.sync_info
            if (type(inst).__name__ != "InstDMACopy"
                    or not si or len(si.on_wait) <= 1):
                continue
            waits = list(si.on_wait)
            keep = [w for w in waits if not w.ant_name.startswith("DMAHW")]
            extras = [w for w in waits if w.ant_name.startswith("DMAHW")]
            if len(keep) + min(1, len(extras)) <= 1:
                continue
            eng = str(inst.engine)
            for earlier in reversed(insts[:idx]):
                if len(keep) + len(extras) <= 1:
                    break
                if str(earlier.engine) != eng:
                    continue
                esi = earlier.sync_info
                if esi is None or esi.on_wait:
                    continue
                esi.on_wait = [extras.pop()]
            si.on_wait = keep + extras
            assert len(si.on_wait) <= 1, (
                f"could not push waits earlier from {inst.name}")


def _spread_adjacent_waits(nc):
    """Move excess waits from a shield ACT/Memset onto the next zero-wait
    instructions of the same engine queue. Queue order keeps the wait ahead
    of every later instruction on that engine, which is exactly the WAR
    ordering the wait protects."""
    for block in nc.m.functions[0].blocks:
        insts = list(block.instructions)
        for idx, inst in enumerate(insts):
            si = inst.sync_info
            if (type(inst).__name__ not in ("InstActivation", "InstMemset")
                    or not si or len(si.on_wait) <= 1):
                continue
            waits = list(si.on_wait)
            extras = waits[1:]
            eng = str(inst.engine)
            for later in insts[idx + 1:idx + 12]:
                if not extras:
                    break
                if str(later.engine) != eng:
                    continue
                lsi = later.sync_info
                if lsi is None or lsi.on_wait:
                    break
                lsi.on_wait = [extras.pop(0)]
            si.on_wait = waits[:1]
            assert not extras, (
                f"could not spread {len(extras)} waits from {inst.name}")


def _split_tail_drain_waits(nc):
    """walrus allows one sync-wait per instruction; the kernel-tail drain can
    carry one wait per outstanding DMA lane. Redistribute the extras onto
    later same-engine drains whose waits are all trivial (value 0)."""
    for block in nc.m.functions[0].blocks:
        insts = list(block.instructions)
        for idx, inst in enumerate(insts):
            si = inst.sync_info
            if type(inst).__name__ != "InstDrain" or not si or len(si.on_wait) <= 1:
                continue
            waits = list(si.on_wait)
            extras = waits[1:]
            si.on_wait = waits[:1]
            eng = str(inst.engine)
            # prefer same-engine drains, then any pre-barrier drain: every
            # engine rendezvouses at the exit barrier, so a DMA-completion
            # wait on any drain still precedes kernel exit.
            for same_engine in (True, False):
                for later in insts[idx + 1:]:
                    if not extras:
                        break
                    lsi = later.sync_info
                    if (type(later).__name__ == "InstDrain"
                            and (str(later.engine) == eng) == same_engine
                            and lsi is not None
                            and all(w.wait_value == 0 for w in lsi.on_wait)):
                        lsi.on_wait = [extras.pop(0)]
            assert not extras, (
                f"could not redistribute {len(extras)} drain waits on {inst.name}")


def _host_prep(x, weight, bias):
    x = np.ascontiguousarray(x, dtype=np.float32)
    weight = np.ascontiguousarray(weight, dtype=np.float32)
    bias = np.ascontiguousarray(bias, dtype=np.float32)

    # zero-padded volume; d axis padded to 70 so the upper core's 38-plane
    # shard (and the ragged block's 8-plane window) stays in-bounds.
    xp = np.zeros((B, CIN, 70, HP, WP), dtype=_NPDT)
    xp[:, :, 1:S + 1, 1:S + 1, 1:S + 1] = x.astype(_NPDT)

    # banded weights: wbd[(cin,dw), t=(kh,kw), (ds,co)]
    wbd = np.zeros((CIN, NDW, 9, 96), dtype=_NPDT)
    wt = weight.astype(_NPDT).transpose(1, 0, 2, 3, 4).reshape(CIN, COUT, 3, 9)
    for ds in range(DSUB):
        for kd in range(3):
            wbd[:, ds + kd, :, ds * 16:(ds + 1) * 16] = wt[:, :, kd, :].transpose(0, 2, 1)
    wbd = np.ascontiguousarray(wbd.reshape(128, 9 * 96))

    bias96 = np.ascontiguousarray(np.tile(bias, DSUB)[:, None])

    in_maps = []
    for core in range(8):
        b, h = divmod(core, 2)
        xsh = np.ascontiguousarray(
            xp[b, :, 32 * h:32 * h + SHARD_D].reshape(CIN, SHARD_D, PLANE))
        in_maps.append({"xs": xsh, "wb": wbd, "bs": bias96})
    return in_maps


def kernel(x, weight, bias):
    global _nc_cache, LAST_RESULT
    if _nc_cache is None:
        _nc_cache = _build_nc()
    nc = _nc_cache

    in_maps = _host_prep(x, weight, bias)
    trace = bool(int(os.environ.get("KERNEL_TRACE", "0")))
    res = run_bass_kernel_spmd(nc, in_maps, core_ids=list(range(8)), trace=trace)
    LAST_RESULT = res

    out = np.empty((B, COUT, S, S, S), dtype=np.float32)
    for core in range(8):
        b, h = divmod(core, 2)
        out[b, :, 32 * h:32 * h + 32] = (
            res.results[core]["out"].astype(np.float32).reshape(COUT, DHALF, S, S))
    return out


# revision 49
# speedup vs baseline: 1.0696x; 1.0696x over previous
"""Trainium2 Bass kernel for 3D conv: x[4,16,64,64,64] * w[16,16,3,3,3] + bias, pad=1.

Strategy (8 cores): shard over (batch, d-half) -> 8 shards of [16, 32+halo, 66, 66].
Per core, a "banded weight" matmul decomposition:
  - contraction K = (cin=16) x (d-window=8) = 128 partitions
  - output    M = (d_sub<=6 outputs) x (cout=16) <= 96 partitions
  - the 9 (kh,kw) taps are free-dim shifts over a zero-padded 66x66 (h,w) plane
  - lhsT[(cin,dw), (ds,co)] = W[co,cin,dw-ds,kh,kw] for 0<=dw-ds<=2 (banded, built on host)
Matmuls run in float32r (full-rate fp32 mode for free dim >= 256).
Bias is fused into the PSUM->SBUF extraction copy on the scalar engine.
"""

import os
from contextlib import ExitStack

import ml_dtypes
import numpy as np

import concourse.bass as bass
import concourse.mybir as mybir
import concourse.tile as tile
from concourse.bass_utils import run_bass_kernel_spmd

USE_BF16 = True            # bf16 x/weights/output (fp32 psum accumulation)
_MMDT = mybir.dt.bfloat16 if USE_BF16 else mybir.dt.float32r
_IODT = mybir.dt.bfloat16 if USE_BF16 else mybir.dt.float32
_NPDT = ml_dtypes.bfloat16 if USE_BF16 else np.float32

B, CIN, COUT, S = 4, 16, 16, 64
HP = WP = S + 2            # padded plane edge: 66
PLANE = HP * WP            # 4356
MARGIN = 68                # free-dim margin so shifted reads stay in-bounds
DSUB = 6                   # d outputs per full block
NDW = 8                    # d-window planes (DSUB + 2 halo)
DHALF = 32                 # output d planes per core
SHARD_D = DHALF + 6        # padded d planes per shard (windows span [-1, 37))
BLOCKS = [(0, 6), (6, 6), (12, 6), (18, 6), (24, 6), (30, 2)]
CROWS = 7                  # padded h-rows per psum chunk (7*66=462 <= 512 bank limit)
OBW = CROWS * S            # 448 output cols per full chunk

_nc_cache = None
LAST_RESULT = None         # BassKernelResults of the most recent run (for test.py)


def _strip_implied_waits(nc):
    """Remove semaphore waits that are transitively implied by another wait on
    the same instruction.

    Tile's add_semaphores emits the full non-transitive closure, so a matmul
    whose psum slot was last touched by (PE writes -> ACT read -> DVE memset)
    carries three waits — but walrus only supports a single sync-wait on a
    Matmult (fp32r matmuls are self-loading, and the wait rides the LDW
    struct). A wait (s >= v) is provably redundant if another wait on the
    same instruction targets a producer whose completion already implies
    (s >= v). We replay the scheduled instruction stream with vector clocks
    to compute each semaphore event's implied clock, then drop implied waits.

    In-order completion is assumed per compute-engine queue but NOT for DMA
    instructions (SDMA engines complete packets out of order), so DMA clocks
    only carry their own waits + update.
    """
    sem_count = {}
    sem_events = {}        # sem id -> list of (value_after, clock dict)
    engine_clock = {}
    engine_self = {}       # engine -> {sem id -> updates issued by that engine}

    def join(a, b):
        for k, v in b.items():
            if a.get(k, -1) < v:
                a[k] = v

    def snapshot(sid, val):
        for value_after, clk in sem_events.get(sid, ()):
            if value_after >= val:
                return clk
        return None

    for block in nc.m.functions[0].blocks:
        for inst in block.instructions:
            si = inst.sync_info
            if si is None:
                continue
            eng0 = str(inst.engine)
            is_dma0 = type(inst).__name__ in ("InstDMACopy", "InstDMATranspose")
            is_serial = (not is_dma0 and type(inst).__name__ not in
                         ("InstMatmult", "InstDrain", "InstEventSemaphore"))
            waits = list(si.on_wait)
            if is_serial and len(waits) > 1:
                # serial engines execute in order: a wait on the engine's own
                # completion semaphore for a value its predecessors already
                # produce is a no-op.
                own = engine_self.get(eng0, {})
                kept = [w for w in waits
                        if not (w.wait_mode == "sem-ge-imm"
                                and own.get(w.id, 0) >= w.wait_value)]
                if len(kept) < len(waits):
                    si.on_wait = kept
                    waits = kept
            snaps = []
            for w in waits:
                snaps.append(snapshot(w.id, w.wait_value)
                             if w.wait_mode == "sem-ge-imm" else None)
            if len(waits) > 1:
                keep = []
                for i, w in enumerate(waits):
                    if w.wait_mode != "sem-ge-imm":
                        keep.append(w)
                        continue
                    implied = False
                    for j, other in enumerate(waits):
                        if i == j or snaps[j] is None:
                            continue
                        if snaps[j].get(w.id, -1) >= w.wait_value:
                            implied = True
                            break
                    if not implied:
                        keep.append(w)
                if len(keep) < len(waits):
                    si.on_wait = keep
                    waits = keep
                    snaps = [snapshot(w.id, w.wait_value)
                             if w.wait_mode == "sem-ge-imm" else None
                             for w in waits]

            clk = {}
            for s in snaps:
                if s is not None:
                    join(clk, s)
            eng = str(inst.engine)
            is_dma = type(inst).__name__ in ("InstDMACopy", "InstDMATranspose")
            if not is_dma and eng in engine_clock:
                join(clk, engine_clock[eng])
            for u in si.on_update:
                if u.update_mode == "sem-add-imm":
                    sem_count[u.id] = sem_count.get(u.id, 0) + u.update_value
                elif u.update_mode == "sem-inc":
                    sem_count[u.id] = sem_count.get(u.id, 0) + 1
                else:
                    continue
                clk[u.id] = max(clk.get(u.id, 0), sem_count[u.id])
                sem_events.setdefault(u.id, []).append((sem_count[u.id], clk))
                if not is_dma:
                    es = engine_self.setdefault(eng, {})
                    es[u.id] = sem_count[u.id]
            if not is_dma:
                engine_clock[eng] = clk


def _build_nc():
    nc = bass.Bass()
    xs = nc.dram_tensor("xs", [CIN, SHARD_D, PLANE], _MMDT,
                        kind="ExternalInput")
    wb = nc.dram_tensor("wb", [128, 9 * 96], _MMDT,
                        kind="ExternalInput")
    bs = nc.dram_tensor("bs", [96, 1], mybir.dt.float32, kind="ExternalInput")
    out = nc.dram_tensor("out", [COUT, DHALF, S * S], _IODT,
                         kind="ExternalOutput")

    with ExitStack() as ctx:
        tc = ctx.enter_context(tile.TileContext(nc))
        consts = ctx.enter_context(tc.tile_pool(name="consts", bufs=1))
        xpool = ctx.enter_context(tc.tile_pool(name="xpool", bufs=6))
        opool = ctx.enter_context(tc.tile_pool(name="opool", bufs=2))
        pspool = ctx.enter_context(tc.tile_pool(name="pspool", bufs=7, space="PSUM"))

        shield = ctx.enter_context(tc.tile_pool(name="shield", bufs=1, space="PSUM"))
        sps = shield.tile([2, 8], mybir.dt.float32)
        ssb = consts.tile([1, 8], mybir.dt.float32)

        wtile = consts.tile([128, 9 * 96], _MMDT)
        nc.sync.dma_start(out=wtile, in_=wb[:, :])
        # prefetch all six block windows up front (SP ring, back to back)
        # split point between the two load halves of each block, in plane cols
        XSPLIT = 24 * WP  # rows [0,24) / [24,66): chunks 0-2 need only half A
        xts = []
        for blk, (dr0, dsc) in enumerate(BLOCKS):
            xt = xpool.tile([128, PLANE + 2 * MARGIN], _MMDT, tag="xt")
            # src iterates (cin, dw, plane) -> partition p = cin*8+dw.
            # All loads on the SP ring: FIFO order matches consumption order.
            # Each block in two halves so early chunks can start sooner.
            nc.sync.dma_start(
                out=xt[:, MARGIN:MARGIN + XSPLIT],
                in_=xs[:, dr0:dr0 + NDW, 0:XSPLIT],
            )
            nc.sync.dma_start(
                out=xt[:, MARGIN + XSPLIT:MARGIN + PLANE],
                in_=xs[:, dr0:dr0 + NDW, XSPLIT:PLANE],
            )
            xts.append(xt)
        btile = consts.tile([96, 1], mybir.dt.float32)
        nc.sync.dma_start(out=btile, in_=bs[:, :])
        # walrus allows only one sync-wait on a Matmult; absorb each DMA's
        # completion wait with a dummy 2x2 PE / 1-elem ACT op reading the tile.
        nc.tensor.matmul(sps[0:2, 0:2], wtile[0:2, 0:2], wtile[0:2, 0:2],
                         start=True, stop=True)
        nc.scalar.activation(ssb[0:1, 0:1], btile[0:1, 0:1],
                             mybir.ActivationFunctionType.Copy)

        for blk, (dr0, dsc) in enumerate(BLOCKS):
            mv = 16 * dsc
            xt = xts[blk]
            # absorb the xt DMA waits (one per load half) on the PE engine
            nc.tensor.matmul(sps[0:2, 2:4], xt[0:2, MARGIN:MARGIN + 2],
                             xt[0:2, MARGIN:MARGIN + 2], start=True, stop=True)
            nc.tensor.matmul(
                sps[0:2, 4:6], xt[0:2, MARGIN + XSPLIT:MARGIN + XSPLIT + 2],
                xt[0:2, MARGIN + XSPLIT:MARGIN + XSPLIT + 2],
                start=True, stop=True)
            ob = opool.tile([96, S * S], _IODT, tag="ob")
            # absorb the ob-slot-release (out DMA) waits on the ACT engine
            # (one per store of the slot's previous user)
            nc.scalar.activation(ob[0:1, 0:1], ssb[0:1, 0:1],
                                 mybir.ActivationFunctionType.Copy)
            nc.scalar.activation(ob[0:1, 2048:2049], ssb[0:1, 1:2],
                                 mybir.ActivationFunctionType.Copy)
            nc.scalar.activation(ob[0:1, 4032:4033], ssb[0:1, 2:3],
                                 mybir.ActivationFunctionType.Copy)
            for c in range(10):
                rc = CROWS if c < 9 else 1
                ncols = WP * rc
                ps = pspool.tile([96, 512], mybir.dt.float32, tag="ps")
                # absorb the psum-slot-release waits on DVE so the chunk's
                # first matmul carries at most one wait
                nc.vector.memset(ps[0:1, 0:1], 0.0)
                base = MARGIN + WP * (1 + CROWS * c)
                for t in range(9):
                    kh, kw = divmod(t, 3)
                    off = base + (kh - 1) * WP + (kw - 1)
                    nc.tensor.matmul(
                        ps[:mv, :ncols],
                        wtile[:, t * 96:t * 96 + mv],
                        xt[:, off:off + ncols],
                        start=(t == 0),
                        stop=(t == 8),
                    )
                if rc > 1:
                    src = ps[:mv, 1:1 + rc * WP].rearrange(
                        "p (r s) -> p r s", r=rc)[:, :, 0:S]
                    dst = ob[:mv, OBW * c:OBW * c + rc * S].rearrange(
                        "p (r s) -> p r s", r=rc)
                else:
                    src = ps[:mv, 1:1 + S]
                    dst = ob[:mv, OBW * c:OBW * c + S]
                nc.scalar.activation(
                    out=dst, in_=src,
                    func=mybir.ActivationFunctionType.Identity,
                    bias=btile[:mv, :],
                )
                if c in (4, 8):
                    # store finished columns as soon as their chunks extract;
                    # dest iterates (ds, co, cols) = partition ds*16+co.
                    # ACT-ring issue: stores never block SP-ring loads.
                    lo, hi = (0, 2048) if c == 4 else (2048, 4032)
                    osl = out[:, dr0:dr0 + dsc, lo:hi]
                    oap = bass.AP(tensor=osl.tensor, offset=osl.offset,
                                  ap=[osl.ap[1], osl.ap[0], osl.ap[2]])
                    nc.scalar.dma_start(out=oap, in_=ob[:mv, lo:hi])
            osl = out[:, dr0:dr0 + dsc, 4032:4096]
            oap = bass.AP(tensor=osl.tensor, offset=osl.offset,
                          ap=[osl.ap[1], osl.ap[0], osl.ap[2]])
            nc.scalar.dma_start(out=oap, in_=ob[:mv, 4032:4096])
    _strip_implied_waits(nc)
    _spread_adjacent_waits(nc)
    _push_waits_earlier(nc)
    _split_tail_drain_waits(nc)
    return nc


def _push_waits_earlier(nc):
    """For a DMACopy still carrying >1 waits (e.g. a DMAHW lane-ordering wait
    plus a data wait), move the extras onto an earlier zero-wait instruction
    of the same engine queue. Satisfying a wait earlier in the queue is
    strictly more conservative — provided the instruction that produces the
    awaited semaphore value is not itself issued later on that queue (which
    would deadlock). Producers are located with a semaphore replay."""
    for block in nc.m.functions[0].blocks:
        insts = list(block.instructions)
        # replay semaphore counts to locate each (sem, value)'s producer idx
        counts = {}
        events = {}  # sem id -> list of (value_after, idx)
        for idx, inst in enumerate(insts):
            si = inst.sync_info
            if not si:
                continue
            for u in si.on_update:
                inc = u.update_value if u.update_mode == "sem-add-imm" else (
                    1 if u.update_mode == "sem-inc" else 0)
                if not inc:
                    continue
                counts[u.id] = counts.get(u.id, 0) + inc
                events.setdefault(u.id, []).append((counts[u.id], idx))

        def producer_idx(sid, val):
            for value_after, idx in events.get(sid, ()):
                if value_after >= val:
                    return idx
            return None

        for idx, inst in enumerate(insts):
            si = inst.sync_info
            if (type(inst).__name__ != "InstDMACopy"
                    or not si or len(si.on_wait) <= 1):
                continue
            waits = list(si.on_wait)
            keep = [w for w in waits if not w.ant_name.startswith("DMAHW")]
            extras = [w for w in waits if w.ant_name.startswith("DMAHW")]
            if len(keep) + min(1, len(extras)) <= 1:
                continue
            eng = str(inst.engine)
            for eidx in range(idx - 1, -1, -1):
                if len(keep) + len(extras) <= 1:
                    break
                earlier = insts[eidx]
                if str(earlier.engine) != eng:
                    continue
                esi = earlier.sync_info
                if esi is None or esi.on_wait:
                    continue
                w = extras[-1]
                p = producer_idx(w.id, w.wait_value)
                if p is None:
                    continue
                prod = insts[p]
                psi = prod.sync_info
                same_queue = str(prod.engine) == eng
                blocked_prod = psi is not None and bool(psi.on_wait)
                if p >= eidx and (same_queue or blocked_prod):
                    continue  # placing here could form a wait cycle
                esi.on_wait = [extras.pop()]
            si.on_wait = keep + extras
            assert len(si.on_wait) <= 1, (
                f"could not push waits earlier from {inst.name}")


def _spread_adjacent_waits(nc):
    """Move excess waits from a shield ACT/Memset onto the next zero-wait
    instructions of the same engine queue. Queue order keeps the wait ahead
    of every later instruction on that engine, which is exactly the WAR
    ordering the wait protects."""
    for block in nc.m.functions[0].blocks:
        insts = list(block.instructions)
        for idx, inst in enumerate(insts):
            si = inst.sync_info
            if (type(inst).__name__ not in ("InstActivation", "InstMemset")
                    or not si or len(si.on_wait) <= 1):
                continue
            waits = list(si.on_wait)
            extras = waits[1:]
            eng = str(inst.engine)
            for later in insts[idx + 1:idx + 12]:
                if not extras:
                    break
                if str(later.engine) != eng:
                    continue
                lsi = later.sync_info
                if lsi is None or lsi.on_wait:
                    break
                lsi.on_wait = [extras.pop(0)]
            si.on_wait = waits[:1]
            assert not extras, (
                f"could not spread {len(extras)} waits from {inst.name}")


def _split_tail_drain_waits(nc):
    """walrus allows one sync-wait per instruction; the kernel-tail drain can
    carry one wait per outstanding DMA lane. Redistribute the extras onto
    later same-engine drains whose waits are all trivial (value 0)."""
    for block in nc.m.functions[0].blocks:
        insts = list(block.instructions)
        for idx, inst in enumerate(insts):
            si = inst.sync_info
            if type(inst).__name__ != "InstDrain" or not si or len(si.on_wait) <= 1:
                continue
            waits = list(si.on_wait)
            extras = waits[1:]
            si.on_wait = waits[:1]
            eng = str(inst.engine)
            # prefer same-engine drains, then any pre-barrier drain: every
            # engine rendezvouses at the exit barrier, so a DMA-completion
            # wait on any drain still precedes kernel exit.
            for same_engine in (True, False):
                for later in insts[idx + 1:]:
                    if not extras:
                        break
                    lsi = later.sync_info
                    if (type(later).__name__ == "InstDrain"
                            and (str(later.engine) == eng) == same_engine
                            and lsi is not None
                            and all(w.wait_value == 0 for w in lsi.on_wait)):
                        lsi.on_wait = [extras.pop(0)]
            assert not extras, (
                f"could not redistribute {len(extras)} drain waits on {inst.name}")


def _host_prep(x, weight, bias):
    x = np.ascontiguousarray(x, dtype=np.float32)
    weight = np.ascontiguousarray(weight, dtype=np.float32)
    bias = np.ascontiguousarray(bias, dtype=np.float32)

    # zero-padded volume; d axis padded to 70 so the upper core's 38-plane
    # shard (and the ragged block's 8-plane window) stays in-bounds.
    xp = np.zeros((B, CIN, 70, HP, WP), dtype=_NPDT)
    xp[:, :, 1:S + 1, 1:S + 1, 1:S + 1] = x.astype(_NPDT)

    # banded weights: wbd[(cin,dw), t=(kh,kw), (ds,co)]
    wbd = np.zeros((CIN, NDW, 9, 96), dtype=_NPDT)
    wt = weight.astype(_NPDT).transpose(1, 0, 2, 3, 4).reshape(CIN, COUT, 3, 9)
    for ds in range(DSUB):
        for kd in range(3):
            wbd[:, ds + kd, :, ds * 16:(ds + 1) * 16] = wt[:, :, kd, :].transpose(0, 2, 1)
    wbd = np.ascontiguousarray(wbd.reshape(128, 9 * 96))

    bias96 = np.ascontiguousarray(np.tile(bias, DSUB)[:, None])

    in_maps = []
    for core in range(8):
        b, h = divmod(core, 2)
        xsh = np.ascontiguousarray(
            xp[b, :, 32 * h:32 * h + SHARD_D].reshape(CIN, SHARD_D, PLANE))
        in_maps.append({"xs": xsh, "wb": wbd, "bs": bias96})
    return in_maps


def kernel(x, weight, bias):
    global _nc_cache, LAST_RESULT
    if _nc_cache is None:
        _nc_cache = _build_nc()
    nc = _nc_cache

    in_maps = _host_prep(x, weight, bias)
    trace = bool(int(os.environ.get("KERNEL_TRACE", "0")))
    res = run_bass_kernel_spmd(nc, in_maps, core_ids=list(range(8)), trace=trace)
    LAST_RESULT = res

    out = np.empty((B, COUT, S, S, S), dtype=np.float32)
    for core in range(8):
        b, h = divmod(core, 2)
        out[b, :, 32 * h:32 * h + 32] = (
            res.results[core]["out"].astype(np.float32).reshape(COUT, DHALF, S, S))
    return out


# revision 52
# speedup vs baseline: 1.0847x; 1.0141x over previous
"""Trainium2 Bass kernel for 3D conv: x[4,16,64,64,64] * w[16,16,3,3,3] + bias, pad=1.

Strategy (8 cores): shard over (batch, d-half) -> 8 shards of [16, 32+halo, 66, 66].
Per core, a "banded weight" matmul decomposition:
  - contraction K = (cin=16) x (d-window=8) = 128 partitions
  - output    M = (d_sub<=6 outputs) x (cout=16) <= 96 partitions
  - the 9 (kh,kw) taps are free-dim shifts over a zero-padded 66x66 (h,w) plane
  - lhsT[(cin,dw), (ds,co)] = W[co,cin,dw-ds,kh,kw] for 0<=dw-ds<=2 (banded, built on host)
Matmuls run in float32r (full-rate fp32 mode for free dim >= 256).
Bias is fused into the PSUM->SBUF extraction copy on the scalar engine.
"""

import os
from contextlib import ExitStack

import ml_dtypes
import numpy as np

import concourse.bass as bass
import concourse.mybir as mybir
import concourse.tile as tile
from concourse.bass_utils import run_bass_kernel_spmd

USE_BF16 = True            # bf16 x/weights/output (fp32 psum accumulation)
_MMDT = mybir.dt.bfloat16 if USE_BF16 else mybir.dt.float32r
_IODT = mybir.dt.bfloat16 if USE_BF16 else mybir.dt.float32
_NPDT = ml_dtypes.bfloat16 if USE_BF16 else np.float32

B, CIN, COUT, S = 4, 16, 16, 64
HP = WP = S + 2            # padded plane edge: 66
PLANE = HP * WP            # 4356
MARGIN = 68                # free-dim margin so shifted reads stay in-bounds
DSUB = 6                   # d outputs per full block
NDW = 8                    # d-window planes (DSUB + 2 halo)
DHALF = 32                 # output d planes per core
SHARD_D = DHALF + 6        # padded d planes per shard (windows span [-1, 37))
BLOCKS = [(0, 6), (6, 6), (12, 6), (18, 6), (24, 6), (30, 2)]
CROWS = 7                  # padded h-rows per psum chunk (7*66=462 <= 512 bank limit)
OBW = CROWS * S            # 448 output cols per full chunk

_nc_cache = None
LAST_RESULT = None         # BassKernelResults of the most recent run (for test.py)


def _strip_implied_waits(nc):
    """Remove semaphore waits that are transitively implied by another wait on
    the same instruction.

    Tile's add_semaphores emits the full non-transitive closure, so a matmul
    whose psum slot was last touched by (PE writes -> ACT read -> DVE memset)
    carries three waits — but walrus only supports a single sync-wait on a
    Matmult (fp32r matmuls are self-loading, and the wait rides the LDW
    struct). A wait (s >= v) is provably redundant if another wait on the
    same instruction targets a producer whose completion already implies
    (s >= v). We replay the scheduled instruction stream with vector clocks
    to compute each semaphore event's implied clock, then drop implied waits.

    In-order completion is assumed per compute-engine queue but NOT for DMA
    instructions (SDMA engines complete packets out of order), so DMA clocks
    only carry their own waits + update.
    """
    sem_count = {}
    sem_events = {}        # sem id -> list of (value_after, clock dict)
    engine_clock = {}
    engine_self = {}       # engine -> {sem id -> updates issued by that engine}

    def join(a, b):
        for k, v in b.items():
            if a.get(k, -1) < v:
                a[k] = v

    def snapshot(sid, val):
        for value_after, clk in sem_events.get(sid, ()):
            if value_after >= val:
                return clk
        return None

    for block in nc.m.functions[0].blocks:
        for inst in block.instructions:
            si = inst.sync_info
            if si is None:
                continue
            eng0 = str(inst.engine)
            is_dma0 = type(inst).__name__ in ("InstDMACopy", "InstDMATranspose")
            is_serial = (not is_dma0 and type(inst).__name__ not in
                         ("InstMatmult", "InstDrain", "InstEventSemaphore"))
            waits = list(si.on_wait)
            if is_serial and len(waits) > 1:
                # serial engines execute in order: a wait on the engine's own
                # completion semaphore for a value its predecessors already
                # produce is a no-op.
                own = engine_self.get(eng0, {})
                kept = [w for w in waits
                        if not (w.wait_mode == "sem-ge-imm"
                                and own.get(w.id, 0) >= w.wait_value)]
                if len(kept) < len(waits):
                    si.on_wait = kept
                    waits = kept
            snaps = []
            for w in waits:
                snaps.append(snapshot(w.id, w.wait_value)
                             if w.wait_mode == "sem-ge-imm" else None)
            if len(waits) > 1:
                keep = []
                for i, w in enumerate(waits):
                    if w.wait_mode != "sem-ge-imm":
                        keep.append(w)
                        continue
                    implied = False
                    for j, other in enumerate(waits):
                        if i == j or snaps[j] is None:
                            continue
                        if snaps[j].get(w.id, -1) >= w.wait_value:
                            implied = True
                            break
                    if not implied:
                        keep.append(w)
                if len(keep) < len(waits):
                    si.on_wait = keep
                    waits = keep
                    snaps = [snapshot(w.id, w.wait_value)
                             if w.wait_mode == "sem-ge-imm" else None
                             for w in waits]

            clk = {}
            for s in snaps:
                if s is not None:
                    join(clk, s)
            eng = str(inst.engine)
            is_dma = type(inst).__name__ in ("InstDMACopy", "InstDMATranspose")
            if not is_dma and eng in engine_clock:
                join(clk, engine_clock[eng])
            for u in si.on_update:
                if u.update_mode == "sem-add-imm":
                    sem_count[u.id] = sem_count.get(u.id, 0) + u.update_value
                elif u.update_mode == "sem-inc":
                    sem_count[u.id] = sem_count.get(u.id, 0) + 1
                else:
                    continue
                clk[u.id] = max(clk.get(u.id, 0), sem_count[u.id])
                sem_events.setdefault(u.id, []).append((sem_count[u.id], clk))
                if not is_dma:
                    es = engine_self.setdefault(eng, {})
                    es[u.id] = sem_count[u.id]
            if not is_dma:
                engine_clock[eng] = clk


def _build_nc():
    nc = bass.Bass()
    xs = nc.dram_tensor("xs", [CIN, SHARD_D, PLANE], _MMDT,
                        kind="ExternalInput")
    wb = nc.dram_tensor("wb", [128, 9 * 96], _MMDT,
                        kind="ExternalInput")
    bs = nc.dram_tensor("bs", [96, 1], mybir.dt.float32, kind="ExternalInput")
    out = nc.dram_tensor("out", [COUT, DHALF, S * S], _IODT,
                         kind="ExternalOutput")

    with ExitStack() as ctx:
        tc = ctx.enter_context(tile.TileContext(nc))
        consts = ctx.enter_context(tc.tile_pool(name="consts", bufs=1))
        xpool = ctx.enter_context(tc.tile_pool(name="xpool", bufs=6))
        opool = ctx.enter_context(tc.tile_pool(name="opool", bufs=2))
        pspool = ctx.enter_context(tc.tile_pool(name="pspool", bufs=7, space="PSUM"))

        shield = ctx.enter_context(tc.tile_pool(name="shield", bufs=1, space="PSUM"))
        sps = shield.tile([2, 8], mybir.dt.float32)
        ssb = consts.tile([1, 8], mybir.dt.float32)

        wtile = consts.tile([128, 9 * 96], _MMDT)
        # first tap's columns first: the opening matmul only needs [0:96]
        nc.sync.dma_start(out=wtile[:, 0:96], in_=wb[:, 0:96])
        nc.sync.dma_start(out=wtile[:, 96:], in_=wb[:, 96:])
        # prefetch all six block windows up front (SP ring, back to back)
        # split point between the two load halves of each block, in plane cols
        XSPLIT = 24 * WP  # rows [0,24) / [24,66): chunks 0-2 need only half A
        xts = []
        for blk, (dr0, dsc) in enumerate(BLOCKS):
            xt = xpool.tile([128, PLANE + 2 * MARGIN], _MMDT, tag="xt")
            # src iterates (cin, dw, plane) -> partition p = cin*8+dw.
            # All loads on the SP ring: FIFO order matches consumption order.
            # Each block in two halves so early chunks can start sooner.
            nc.sync.dma_start(
                out=xt[:, MARGIN:MARGIN + XSPLIT],
                in_=xs[:, dr0:dr0 + NDW, 0:XSPLIT],
            )
            nc.sync.dma_start(
                out=xt[:, MARGIN + XSPLIT:MARGIN + PLANE],
                in_=xs[:, dr0:dr0 + NDW, XSPLIT:PLANE],
            )
            xts.append(xt)
        btile = consts.tile([96, 1], mybir.dt.float32)
        nc.sync.dma_start(out=btile, in_=bs[:, :])
        # walrus allows only one sync-wait on a Matmult; absorb each DMA's
        # completion wait with a dummy 2x2 PE / 1-elem ACT op reading the tile.
        nc.tensor.matmul(sps[0:2, 0:2], wtile[0:2, 0:2], wtile[0:2, 0:2],
                         start=True, stop=True)
        nc.scalar.activation(ssb[0:1, 0:1], btile[0:1, 0:1],
                             mybir.ActivationFunctionType.Copy)

        for blk, (dr0, dsc) in enumerate(BLOCKS):
            mv = 16 * dsc
            xt = xts[blk]
            # absorb the xt DMA waits (one per load half) on the PE engine
            nc.tensor.matmul(sps[0:2, 2:4], xt[0:2, MARGIN:MARGIN + 2],
                             xt[0:2, MARGIN:MARGIN + 2], start=True, stop=True)
            nc.tensor.matmul(
                sps[0:2, 4:6], xt[0:2, MARGIN + XSPLIT:MARGIN + XSPLIT + 2],
                xt[0:2, MARGIN + XSPLIT:MARGIN + XSPLIT + 2],
                start=True, stop=True)
            ob = opool.tile([96, S * S], _IODT, tag="ob")
            # absorb the ob-slot-release (out DMA) waits on the ACT engine
            # (one per store of the slot's previous user)
            nc.scalar.activation(ob[0:1, 0:1], ssb[0:1, 0:1],
                                 mybir.ActivationFunctionType.Copy)
            nc.scalar.activation(ob[0:1, 2048:2049], ssb[0:1, 1:2],
                                 mybir.ActivationFunctionType.Copy)
            nc.scalar.activation(ob[0:1, 4032:4033], ssb[0:1, 2:3],
                                 mybir.ActivationFunctionType.Copy)
            spi = 3
            for c in range(10):
                rc = CROWS if c < 9 else 1
                ncols = WP * rc
                ps = pspool.tile([96, 512], mybir.dt.float32, tag="ps")
                # absorb the psum-slot-release waits on DVE so the chunk's
                # first matmul carries at most one wait
                nc.vector.memset(ps[0:1, 0:1], 0.0)
                base = MARGIN + WP * (1 + CROWS * c)
                for t in range(9):
                    kh, kw = divmod(t, 3)
                    off = base + (kh - 1) * WP + (kw - 1)
                    nc.tensor.matmul(
                        ps[:mv, :ncols],
                        wtile[:, t * 96:t * 96 + mv],
                        xt[:, off:off + ncols],
                        start=(t == 0),
                        stop=(t == 8),
                    )
                if rc > 1:
                    src = ps[:mv, 1:1 + rc * WP].rearrange(
                        "p (r s) -> p r s", r=rc)[:, :, 0:S]
                    dst = ob[:mv, OBW * c:OBW * c + rc * S].rearrange(
                        "p (r s) -> p r s", r=rc)
                else:
                    src = ps[:mv, 1:1 + S]
                    dst = ob[:mv, OBW * c:OBW * c + S]
                nc.scalar.activation(
                    out=dst, in_=src,
                    func=mybir.ActivationFunctionType.Identity,
                    bias=btile[:mv, :],
                )
                if c in (4, 8):
                    # store finished columns as soon as their chunks extract;
                    # dest iterates (ds, co, cols) = partition ds*16+co.
                    # ACT-ring issue: stores never block SP-ring loads. The
                    # zero-wait sponge right before each store catches pushed
                    # DMA-lane waits without stalling extractions.
                    nc.scalar.activation(ssb[0:1, spi:spi + 1], ssb[0:1, 0:1],
                                         mybir.ActivationFunctionType.Copy)
                    spi += 1
                    lo, hi = (0, 2048) if c == 4 else (2048, 4032)
                    osl = out[:, dr0:dr0 + dsc, lo:hi]
                    oap = bass.AP(tensor=osl.tensor, offset=osl.offset,
                                  ap=[osl.ap[1], osl.ap[0], osl.ap[2]])
                    nc.scalar.dma_start(out=oap, in_=ob[:mv, lo:hi])
            nc.scalar.activation(ssb[0:1, spi:spi + 1], ssb[0:1, 0:1],
                                 mybir.ActivationFunctionType.Copy)
            osl = out[:, dr0:dr0 + dsc, 4032:4096]
            oap = bass.AP(tensor=osl.tensor, offset=osl.offset,
                          ap=[osl.ap[1], osl.ap[0], osl.ap[2]])
            nc.scalar.dma_start(out=oap, in_=ob[:mv, 4032:4096])
    _strip_implied_waits(nc)
    _spread_adjacent_waits(nc)
    _push_waits_earlier(nc)
    _split_tail_drain_waits(nc)
    return nc


def _push_waits_earlier(nc):
    """For a DMACopy still carrying >1 waits (e.g. a DMAHW lane-ordering wait
    plus a data wait), move the extras onto an earlier zero-wait instruction
    of the same engine queue. Satisfying a wait earlier in the queue is
    strictly more conservative — provided the instruction that produces the
    awaited semaphore value is not itself issued later on that queue (which
    would deadlock). Producers are located with a semaphore replay."""
    for block in nc.m.functions[0].blocks:
        insts = list(block.instructions)
        # replay semaphore counts to locate each (sem, value)'s producer idx
        counts = {}
        events = {}  # sem id -> list of (value_after, idx)
        for idx, inst in enumerate(insts):
            si = inst.sync_info
            if not si:
                continue
            for u in si.on_update:
                inc = u.update_value if u.update_mode == "sem-add-imm" else (
                    1 if u.update_mode == "sem-inc" else 0)
                if not inc:
                    continue
                counts[u.id] = counts.get(u.id, 0) + inc
                events.setdefault(u.id, []).append((counts[u.id], idx))

        def producer_idx(sid, val):
            for value_after, idx in events.get(sid, ()):
                if value_after >= val:
                    return idx
            return None

        for idx, inst in enumerate(insts):
            si = inst.sync_info
            if (type(inst).__name__ != "InstDMACopy"
                    or not si or len(si.on_wait) <= 1):
                continue
            waits = list(si.on_wait)
            keep = [w for w in waits if not w.ant_name.startswith("DMAHW")]
            extras = [w for w in waits if w.ant_name.startswith("DMAHW")]
            if len(keep) + min(1, len(extras)) <= 1:
                continue
            eng = str(inst.engine)
            for eidx in range(idx - 1, -1, -1):
                if len(keep) + len(extras) <= 1:
                    break
                earlier = insts[eidx]
                if str(earlier.engine) != eng:
                    continue
                esi = earlier.sync_info
                if esi is None or esi.on_wait:
                    continue
                w = extras[-1]
                p = producer_idx(w.id, w.wait_value)
                if p is None:
                    continue
                prod = insts[p]
                psi = prod.sync_info
                same_queue = str(prod.engine) == eng
                blocked_prod = psi is not None and bool(psi.on_wait)
                if p >= eidx and (same_queue or blocked_prod):
                    continue  # placing here could form a wait cycle
                esi.on_wait = [extras.pop()]
            si.on_wait = keep + extras
            assert len(si.on_wait) <= 1, (
                f"could not push waits earlier from {inst.name}")


def _spread_adjacent_waits(nc):
    """Move excess waits from a shield ACT/Memset onto the next zero-wait
    instructions of the same engine queue. Queue order keeps the wait ahead
    of every later instruction on that engine, which is exactly the WAR
    ordering the wait protects."""
    for block in nc.m.functions[0].blocks:
        insts = list(block.instructions)
        for idx, inst in enumerate(insts):
            si = inst.sync_info
            if (type(inst).__name__ not in ("InstActivation", "InstMemset")
                    or not si or len(si.on_wait) <= 1):
                continue
            waits = list(si.on_wait)
            extras = waits[1:]
            eng = str(inst.engine)
            for later in insts[idx + 1:idx + 12]:
                if not extras:
                    break
                if str(later.engine) != eng:
                    continue
                lsi = later.sync_info
                if lsi is None or lsi.on_wait:
                    break
                lsi.on_wait = [extras.pop(0)]
            si.on_wait = waits[:1]
            assert not extras, (
                f"could not spread {len(extras)} waits from {inst.name}")


def _split_tail_drain_waits(nc):
    """walrus allows one sync-wait per instruction; the kernel-tail drain can
    carry one wait per outstanding DMA lane. Redistribute the extras onto
    later same-engine drains whose waits are all trivial (value 0)."""
    for block in nc.m.functions[0].blocks:
        insts = list(block.instructions)
        for idx, inst in enumerate(insts):
            si = inst.sync_info
            if type(inst).__name__ != "InstDrain" or not si or len(si.on_wait) <= 1:
                continue
            waits = list(si.on_wait)
            extras = waits[1:]
            si.on_wait = waits[:1]
            eng = str(inst.engine)
            # prefer same-engine drains, then any pre-barrier drain: every
            # engine rendezvouses at the exit barrier, so a DMA-completion
            # wait on any drain still precedes kernel exit.
            for same_engine in (True, False):
                for later in insts[idx + 1:]:
                    if not extras:
                        break
                    lsi = later.sync_info
                    if (type(later).__name__ == "InstDrain"
                            and (str(later.engine) == eng) == same_engine
                            and lsi is not None
                            and all(w.wait_value == 0 for w in lsi.on_wait)):
                        lsi.on_wait = [extras.pop(0)]
            assert not extras, (
                f"could not redistribute {len(extras)} drain waits on {inst.name}")


def _host_prep(x, weight, bias):
    x = np.ascontiguousarray(x, dtype=np.float32)
    weight = np.ascontiguousarray(weight, dtype=np.float32)
    bias = np.ascontiguousarray(bias, dtype=np.float32)

    # zero-padded volume; d axis padded to 70 so the upper core's 38-plane
    # shard (and the ragged block's 8-plane window) stays in-bounds.
    xp = np.zeros((B, CIN, 70, HP, WP), dtype=_NPDT)
    xp[:, :, 1:S + 1, 1:S + 1, 1:S + 1] = x.astype(_NPDT)

    # banded weights: wbd[(cin,dw), t=(kh,kw), (ds,co)]
    wbd = np.zeros((CIN, NDW, 9, 96), dtype=_NPDT)
    wt = weight.astype(_NPDT).transpose(1, 0, 2, 3, 4).reshape(CIN, COUT, 3, 9)
    for ds in range(DSUB):
        for kd in range(3):
            wbd[:, ds + kd, :, ds * 16:(ds + 1) * 16] = wt[:, :, kd, :].transpose(0, 2, 1)
    wbd = np.ascontiguousarray(wbd.reshape(128, 9 * 96))

    bias96 = np.ascontiguousarray(np.tile(bias, DSUB)[:, None])

    in_maps = []
    for core in range(8):
        b, h = divmod(core, 2)
        xsh = np.ascontiguousarray(
            xp[b, :, 32 * h:32 * h + SHARD_D].reshape(CIN, SHARD_D, PLANE))
        in_maps.append({"xs": xsh, "wb": wbd, "bs": bias96})
    return in_maps


def kernel(x, weight, bias):
    global _nc_cache, LAST_RESULT
    if _nc_cache is None:
        _nc_cache = _build_nc()
    nc = _nc_cache

    in_maps = _host_prep(x, weight, bias)
    trace = bool(int(os.environ.get("KERNEL_TRACE", "0")))
    res = run_bass_kernel_spmd(nc, in_maps, core_ids=list(range(8)), trace=trace)
    LAST_RESULT = res

    out = np.empty((B, COUT, S, S, S), dtype=np.float32)
    for core in range(8):
        b, h = divmod(core, 2)
        out[b, :, 32 * h:32 * h + 32] = (
            res.results[core]["out"].astype(np.float32).reshape(COUT, DHALF, S, S))
    return out
